# revision 8
# baseline (speedup 1.0000x reference)
"""GCN (3-layer, PyG GCNConv-style) forward on 8 Trainium2 NeuronCores.

Strategy: data-parallel over the 64 graphs (8 graphs per core).  The
message-passing scatter-add is a dense normalized-adjacency matmul run in
fp8e4m3 with MatmulPerfMode.DoubleRow (two 128-row k-tiles per instruction,
0.5 cycles/row), which is 4x the fp32r FLOP rate for the dominant A@h
product.  Weight-side matmuls stay bf16 (weight quantization error is
systematic across nodes and does not average out; fp8 weights blow the
error budget, bf16 lands ~1.6e-3 on the logits).

Host-side prep: the feature gather from the 500k-row table and the dense
A^T build happen on the host; the device receives per-graph feature tiles
(bf16, feature-major [128, 2048]) and A^T tiles (fp8, [128 src-part,
16 src-chunk, 2048 dst] swizzle) so each graph needs exactly two large
contiguous DMAs (features on the Pool DGE queue, A^T on SP so their
fixed per-DMA delays overlap).  Per layer on device:
    h   = x @ W        (16 bf16 matmuls, 4-chunk PSUM groups, bulk-cast
                        to fp8 on DVE)
    x'  = relu(A @ h + b)  (32 fp8 DoubleRow matmuls into 4 psum strips,
                        relu+bias on ACT writing bf16)
The layer orientations alternate (feat-major <-> node-major) so no
transposes are needed anywhere.  The device emits only the per-graph
node-sum accumulators (ACT accum_out); the 64x2 logits head and the
log_softmax run on the host.
"""

import os
import sys

for _p in ("/opt/trn_rl_repo", "/root/.axon_site/_ro/trn_rl_repo"):
    if os.path.isdir(_p) and _p not in sys.path:
        sys.path.insert(0, _p)

import numpy as np
import ml_dtypes

import concourse.bass as bass
import concourse.bacc as bacc
import concourse.mybir as mybir
import concourse.tile as tile
from concourse import bass2jax

G, N, E = 64, 2048, 32768
D = H = 128
O = 2
ALL = 500_000
P = 128
N_CORES = 8
GPC = G // N_CORES          # graphs per core
NCH = N // P                # 128-row chunks per graph (16)

f32 = mybir.dt.float32
bf16 = mybir.dt.bfloat16
f8 = mybir.dt.float8e4

E4NP = ml_dtypes.float8_e4m3      # == mybir.dt.np(float8e4)
BFNP = ml_dtypes.bfloat16

DR = mybir.MatmulPerfMode.DoubleRow
RELU = mybir.ActivationFunctionType.Relu


def _build_program(n_layers: int):
    nc = bacc.Bacc("TRN2", target_bir_lowering=False, debug=False,
                   num_devices=N_CORES)

    NW = 2 + n_layers           # packed bf16 weight blocks: wres, gw[l]s, wfc
    x0 = nc.dram_tensor("x0", [GPC * P, N], bf16, kind="ExternalInput")
    at = nc.dram_tensor("at", [GPC * P, NCH * N], f8, kind="ExternalInput")
    wpk = nc.dram_tensor("wpk", [P, NW * H], bf16, kind="ExternalInput")
    bpk = nc.dram_tensor("bpk", [P, NW], f32, kind="ExternalInput")
    macc_out = nc.dram_tensor("macc_out", [P, GPC * 4], f32,
                              kind="ExternalOutput")

    with tile.TileContext(nc) as tc:
        with tc.tile_pool(name="const", bufs=1) as const, \
             tc.tile_pool(name="apool", bufs=2) as apool, \
             tc.tile_pool(name="inpool", bufs=2) as inpool, \
             tc.tile_pool(name="xpool", bufs=2) as xpool, \
             tc.tile_pool(name="x1pool", bufs=2) as x1pool, \
             tc.tile_pool(name="hpool", bufs=2) as hpool, \
             tc.tile_pool(name="fpool", bufs=2) as fpool, \
             tc.tile_pool(name="hps", bufs=2, space="PSUM") as hps, \
             tc.tile_pool(name="rps", bufs=1, space="PSUM") as rps, \
             tc.tile_pool(name="fps", bufs=1, space="PSUM") as fps, \
             tc.tile_pool(name="aps", bufs=1, space="PSUM") as aps:

            # ---- constants: two packed DMAs ----
            wpk_sb = const.tile([P, NW * H], bf16)
            nc.sync.dma_start(out=wpk_sb[:], in_=wpk[:])
            bpk_sb = const.tile([P, NW], f32)
            nc.sync.dma_start(out=bpk_sb[:], in_=bpk[:])
            wres_sb = wpk_sb[:, 0:H]
            wfc_sb = wpk_sb[:, (NW - 1) * H:NW * H]
            bres_sb = bpk_sb[:, 0:1]
            bfc_sb = bpk_sb[:, NW - 1:NW]
            macc = const.tile([P, GPC * 4], f32)

            def dma_inputs(g):
                """Features via Pool DGE queue, A^T via SP (overlapping
                per-DMA fixed delays).  Graph 0's A^T is quartered so its
                first DoubleRow pairs can start before the full 4MB lands."""
                xT = inpool.tile([P, N], bf16, tag="xin", name=f"x0_{g}")
                nc.gpsimd.dma_start(out=xT[:], in_=x0[g * P:(g + 1) * P, :])
                if g == 0:
                    ats = []
                    for qq in range(4):
                        t = apool.tile([P, 4, N], f8, tag=f"atq{qq}",
                                       name=f"at0_{qq}")
                        nc.sync.dma_start(
                            out=t[:],
                            in_=at[0:P, qq * 4 * N:(qq + 1) * 4 * N].rearrange(
                                "p (s n) -> p s n", s=4))
                        ats.append(t)
                else:
                    t = apool.tile([P, NCH, N], f8, tag="at", name=f"at{g}")
                    nc.sync.dma_start(
                        out=t[:],
                        in_=at[g * P:(g + 1) * P, :].rearrange(
                            "p (s n) -> p s n", s=NCH))
                    ats = [t]
                return xT, ats

            def at_pair(ats, j, q):
                if len(ats) == 4:
                    t, jj = ats[j // 2], (j % 2) * 2
                else:
                    t, jj = ats[0], 2 * j
                return t[:, jj:jj + 2, q * 512:(q + 1) * 512]

            def emit_res_q(g, xT, x1T, q):
                """Residual strip q for graph g (PE filler work)."""
                rp = rps.tile([P, 512], f32, tag="rps", name=f"rp{g}_{q}")
                nc.tensor.matmul(out=rp[:], lhsT=wres_sb,
                                 rhs=xT[:, q * 512:(q + 1) * 512],
                                 start=True, stop=True)
                nc.scalar.activation(
                    out=x1T[:, q * 512:(q + 1) * 512], in_=rp[:],
                    func=RELU, bias=bres_sb)

            def emit_hgrp_q(g, l, x_src, h8t, q):
                """h-group q of layer l (chunks 4q..4q+3) + fp8 bulk cast."""
                hp = hps.tile([P, 512], f32, tag="hps", name=f"hp{g}_{l}_{q}")
                for c in range(4):
                    j = q * 4 + c
                    nc.tensor.matmul(
                        out=hp[:, c * H:(c + 1) * H],
                        lhsT=x_src[:, j * P:(j + 1) * P],
                        rhs=wpk_sb[:, (1 + l) * H:(2 + l) * H],
                        start=(c == 0), stop=(c == 3))
                nc.vector.tensor_copy(
                    out=h8t[:, q * 4:(q + 1) * 4, :].rearrange(
                        "p s f -> p (s f)"),
                    in_=hp[:])

            # ---- graph 0 prologue: inputs + residual + layer-0 h ----
            xT, ats = dma_inputs(0)
            x1T = x1pool.tile([P, N], bf16, tag="x1", name="x1_0")
            h8 = hpool.tile([P, NCH, H], f8, tag="h", name="h0_0")
            for q in range(4):
                emit_res_q(0, xT, x1T, q)
                emit_hgrp_q(0, 0, xT, h8, q)

            for g in range(GPC):
                if g + 1 < GPC:
                    xT_n, ats_n = dma_inputs(g + 1)
                for l in range(n_layers):
                    last = (l == n_layers - 1)
                    ps_l = [aps.tile([P, 512], f32, tag=f"aps{q}",
                                     name=f"as{g}_{l}_{q}") for q in range(4)]
                    # pairs 0-5 interleaved across strips: tolerates the
                    # trailing h-casts of this layer still landing
                    for j in range(6):
                        hj = h8[:, 2 * j:2 * j + 2, :]
                        for q in range(4):
                            nc.tensor.matmul(
                                out=ps_l[q][:], lhsT=hj,
                                rhs=at_pair(ats, j, q),
                                start=(j == 0), stop=False, perf_mode=DR)
                    # finish strips one at a time; each strip's relu output
                    # immediately feeds next-layer h (or fc + next-graph
                    # prologue filler on the last layer)
                    xn = xpool.tile([P, N], bf16, tag="x", name=f"x{g}_{l}")
                    if not last:
                        h8n = hpool.tile([P, NCH, H], f8, tag="h",
                                         name=f"h{g}_{l + 1}")
                    elif g + 1 < GPC:
                        x1T_n = x1pool.tile([P, N], bf16, tag="x1",
                                            name=f"x1_{g + 1}")
                        h8n = hpool.tile([P, NCH, H], f8, tag="h",
                                         name=f"h{g + 1}_0")
                    for q in range(4):
                        for j in (6, 7):
                            nc.tensor.matmul(
                                out=ps_l[q][:],
                                lhsT=h8[:, 2 * j:2 * j + 2, :],
                                rhs=at_pair(ats, j, q),
                                start=False, stop=(j == 7), perf_mode=DR)
                        nc.scalar.activation(
                            out=xn[:, q * 512:(q + 1) * 512], in_=ps_l[q][:],
                            func=RELU, bias=bpk_sb[:, 1 + l:2 + l])
                        if not last:
                            emit_hgrp_q(g, l + 1, xn, h8n, q)
                        else:
                            if g + 1 < GPC:
                                emit_res_q(g + 1, xT_n, x1T_n, q)
                                emit_hgrp_q(g + 1, 0, xT_n, h8n, q)
                            fp = fps.tile([P, 512], f32, tag="fps",
                                          name=f"fp{g}_{q}")
                            nc.tensor.matmul(
                                out=fp[:], lhsT=wfc_sb,
                                rhs=xn[:, q * 512:(q + 1) * 512],
                                start=True, stop=False)
                            nc.tensor.matmul(
                                out=fp[:], lhsT=wfc_sb,
                                rhs=x1T[:, q * 512:(q + 1) * 512],
                                start=False, stop=True)
                            fcq = fpool.tile([P, 512], f32, tag="fcq",
                                             name=f"fc{g}_{q}")
                            nc.scalar.activation(
                                out=fcq[:], in_=fp[:],
                                func=RELU, bias=bfc_sb,
                                accum_out=macc[:, g * 4 + q:g * 4 + q + 1])
                    h8 = h8n
                if g + 1 < GPC:
                    xT, ats, x1T = xT_n, ats_n, x1T_n

            nc.sync.dma_start(out=macc_out[:], in_=macc[:])

    nc.compile()
    return nc


class _Runner:
    """Compile once, keep the jitted sharded executable for repeat calls."""

    def __init__(self, n_layers: int):
        import jax
        from jax.sharding import Mesh, PartitionSpec
        from jax.experimental.shard_map import shard_map

        self.jax = jax
        nc = _build_program(n_layers)
        self.nc = nc
        bass2jax.install_neuronx_cc_hook()

        in_names, out_names, out_avals, zero_outs = [], [], [], []
        pid_name = nc.partition_id_tensor.name if nc.partition_id_tensor else None
        for alloc in nc.m.functions[0].allocations:
            if not isinstance(alloc, mybir.MemoryLocationSet):
                continue
            name = alloc.memorylocations[0].name
            if alloc.kind == "ExternalInput":
                if name != pid_name:
                    in_names.append(name)
            elif alloc.kind == "ExternalOutput":
                out_names.append(name)
                shape = tuple(alloc.tensor_shape)
                dtype = mybir.dt.np(alloc.dtype)
                out_avals.append(jax.core.ShapedArray(shape, dtype))
                zero_outs.append(np.zeros(shape, dtype))
        self.in_names = list(in_names)
        self.out_names = out_names
        self.zero_outs = zero_outs
        n_params = len(in_names)
        all_names = in_names + out_names + ([pid_name] if pid_name else [])

        def _body(*args):
            operands = list(args)
            if pid_name is not None:
                operands.append(bass2jax.partition_id_tensor())
            return tuple(bass2jax._bass_exec_p.bind(
                *operands,
                out_avals=tuple(out_avals),
                in_names=tuple(all_names),
                out_names=tuple(out_names),
                lowering_input_output_aliases=(),
                sim_require_finite=True,
                sim_require_nnan=True,
                nc=nc,
            ))

        devices = jax.devices()[:N_CORES]
        mesh = Mesh(np.asarray(devices), ("core",))
        self.fn = jax.jit(
            shard_map(_body, mesh=mesh,
                      in_specs=(PartitionSpec("core"),) * (n_params + len(out_names)),
                      out_specs=(PartitionSpec("core"),) * len(out_names),
                      check_rep=False),
            keep_unused=True)

    def run(self, concat_inputs: list[np.ndarray]):
        jax = self.jax
        concat_zeros = [np.zeros((N_CORES * z.shape[0], *z.shape[1:]), z.dtype)
                        for z in self.zero_outs]
        outs = self.fn(*concat_inputs, *concat_zeros)
        jax.block_until_ready(outs)
        return {name: np.asarray(outs[i]) for i, name in enumerate(self.out_names)}


_RUNNERS: dict[int, _Runner] = {}


def _prepare_inputs(all_features, feature_index, edge_index,
                    lin_res_w, lin_res_b, gcn_w, gcn_b,
                    fc1_w, fc1_b, lin_w, lin_b, n_layers):
    """Build the concatenated (over cores, axis 0) device input list."""
    feats = np.asarray(all_features, np.float32)
    fi = np.asarray(feature_index).astype(np.int64)
    ei = np.asarray(edge_index).astype(np.int32)

    # host gather + transpose to feature-major bf16 [G, 128, 2048]
    x0_all = np.ascontiguousarray(
        feats[fi].transpose(0, 2, 1)).astype(BFNP)          # [G, D, N]

    # A^T per graph: accumulate duplicate (src,dst) cells, quantize fp8,
    # swizzle to [128 part, 16 chunk, 2048 dst].
    at_all = np.zeros((G, N * N), np.float32)
    diag_keys = (np.arange(N, dtype=np.int64) * (N + 1)).astype(np.int32)
    for g in range(G):
        src = ei[g, 0]
        dst = ei[g, 1]
        deg = np.bincount(dst, minlength=N).astype(np.float32) + 1.0
        dinv = 1.0 / np.sqrt(deg)
        coef = dinv[src] * dinv[dst]
        keys = np.concatenate([src.astype(np.int32) * N + dst, diag_keys])
        vals = np.concatenate([coef, dinv * dinv]).astype(np.float64)
        order = np.argsort(keys, kind="stable")
        ks, vs = keys[order], vals[order]
        first = np.empty(len(ks), bool)
        first[0] = True
        first[1:] = ks[1:] != ks[:-1]
        starts = np.nonzero(first)[0]
        sums = np.add.reduceat(vs, starts).astype(np.float32)
        np.put(at_all[g], ks[starts], sums)
    at8 = at_all.reshape(G, NCH, P, N).transpose(0, 2, 1, 3)  # [G,128,16,2048]
    at8 = np.ascontiguousarray(at8).astype(E4NP).reshape(G, P, NCH * N)

    # packed weights [128, (2+L)*128] bf16: wres | gw[0..L) | wfc
    NW = 2 + n_layers
    wpk = np.empty((P, NW * H), BFNP)
    wpk[:, 0:H] = np.asarray(lin_res_w, np.float32).astype(BFNP)
    for l in range(n_layers):
        wpk[:, (1 + l) * H:(2 + l) * H] = (
            np.asarray(gcn_w[l], np.float32).astype(BFNP))
    wpk[:, (NW - 1) * H:] = np.asarray(fc1_w, np.float32).astype(BFNP)
    # packed biases [128, 2+L] f32: bres | gb[0..L) | bfc
    bpk = np.empty((P, NW), np.float32)
    bpk[:, 0] = np.asarray(lin_res_b, np.float32)
    for l in range(n_layers):
        bpk[:, 1 + l] = np.asarray(gcn_b[l], np.float32)
    bpk[:, NW - 1] = np.asarray(fc1_b, np.float32)

    per_core = {}
    per_core["x0"] = [x0_all[c * GPC:(c + 1) * GPC].reshape(GPC * P, N)
                      for c in range(N_CORES)]
    per_core["at"] = [at8[c * GPC:(c + 1) * GPC].reshape(GPC * P, NCH * N)
                      for c in range(N_CORES)]
    per_core["wpk"] = [wpk] * N_CORES
    per_core["bpk"] = [bpk] * N_CORES
    return per_core


def kernel(all_features, feature_index, edge_index, action,
           lin_res_w, lin_res_b, gcn_w, gcn_b,
           fc1_w, fc1_b, lin_w, lin_b):
    n_layers = int(action) + 1
    assert 1 <= n_layers <= 3

    if n_layers not in _RUNNERS:
        _RUNNERS[n_layers] = _Runner(n_layers)
    runner = _RUNNERS[n_layers]

    per_core = _prepare_inputs(
        all_features, feature_index, edge_index,
        lin_res_w, lin_res_b, gcn_w, gcn_b, fc1_w, fc1_b, lin_w, lin_b,
        n_layers)

    concat = [np.concatenate(per_core[name], axis=0)
              for name in runner.in_names]
    outs = runner.run(concat)

    # host head: node-sums -> means -> logits -> log_softmax
    macc = outs["macc_out"].reshape(N_CORES, P, GPC, 4)
    means = macc.sum(axis=3).transpose(0, 2, 1).reshape(G, H) / N   # [G, H]
    lg = means @ np.asarray(lin_w, np.float32) + np.asarray(lin_b, np.float32)
    mx = lg.max(axis=1, keepdims=True)
    ls = lg - mx - np.log(np.exp(lg - mx).sum(axis=1, keepdims=True))
    return np.asarray(ls, np.float32), np.asarray(lg, np.float32)


# revision 9
# speedup vs baseline: 1.1656x; 1.1656x over previous
"""GCN (3-layer, PyG GCNConv-style) forward on 8 Trainium2 NeuronCores.

Strategy: data-parallel over the 64 graphs (8 graphs per core).  The
message-passing scatter-add is a dense normalized-adjacency matmul run in
fp8e4m3 with MatmulPerfMode.DoubleRow (two 128-row k-tiles per instruction,
0.5 cycles/row), which is 4x the fp32r FLOP rate for the dominant A@h
product.  Weight-side matmuls stay bf16 (weight quantization error is
systematic across nodes and does not average out; fp8 weights blow the
error budget, bf16 lands ~1.6e-3 on the logits).

Host-side prep: the feature gather from the 500k-row table and the dense
A^T build happen on the host; the device receives per-graph feature tiles
(bf16, feature-major [128, 2048]) and A^T tiles (fp8, [128 src-part,
16 src-chunk, 2048 dst] swizzle) so each graph needs exactly two large
contiguous DMAs (features on the Pool DGE queue, A^T on SP so their
fixed per-DMA delays overlap).  Per layer on device:
    h   = x @ W        (16 bf16 matmuls, 4-chunk PSUM groups, bulk-cast
                        to fp8 on DVE)
    x'  = relu(A @ h + b)  (32 fp8 DoubleRow matmuls into 4 psum strips,
                        relu+bias on ACT writing bf16)
The layer orientations alternate (feat-major <-> node-major) so no
transposes are needed anywhere.  The device emits only the per-graph
node-sum accumulators (ACT accum_out); the 64x2 logits head and the
log_softmax run on the host.
"""

import os
import sys

for _p in ("/opt/trn_rl_repo", "/root/.axon_site/_ro/trn_rl_repo"):
    if os.path.isdir(_p) and _p not in sys.path:
        sys.path.insert(0, _p)

import numpy as np
import ml_dtypes

import concourse.bass as bass
import concourse.bacc as bacc
import concourse.mybir as mybir
import concourse.tile as tile
from concourse import bass2jax

G, N, E = 64, 2048, 32768
D = H = 128
O = 2
ALL = 500_000
P = 128
N_CORES = 8
GPC = G // N_CORES          # graphs per core
NCH = N // P                # 128-row chunks per graph (16)

f32 = mybir.dt.float32
bf16 = mybir.dt.bfloat16
f8 = mybir.dt.float8e4

E4NP = ml_dtypes.float8_e4m3      # == mybir.dt.np(float8e4)
BFNP = ml_dtypes.bfloat16

DR = mybir.MatmulPerfMode.DoubleRow
RELU = mybir.ActivationFunctionType.Relu


def _build_program(n_layers: int):
    nc = bacc.Bacc("TRN2", target_bir_lowering=False, debug=False,
                   num_devices=N_CORES)

    NW = 2 + n_layers           # packed bf16 weight blocks: wres, gw[l]s, wfc
    x0 = nc.dram_tensor("x0", [GPC * P, N], bf16, kind="ExternalInput")
    at = nc.dram_tensor("at", [GPC * P, NCH * N], f8, kind="ExternalInput")
    wpk = nc.dram_tensor("wpk", [P, NW * H], bf16, kind="ExternalInput")
    bpk = nc.dram_tensor("bpk", [P, NW], f32, kind="ExternalInput")
    macc_out = nc.dram_tensor("macc_out", [P, GPC * 4], f32,
                              kind="ExternalOutput")

    with tile.TileContext(nc) as tc:
        with tc.tile_pool(name="const", bufs=1) as const, \
             tc.tile_pool(name="apool", bufs=2) as apool, \
             tc.tile_pool(name="inpool", bufs=2) as inpool, \
             tc.tile_pool(name="xpool", bufs=2) as xpool, \
             tc.tile_pool(name="x1pool", bufs=2) as x1pool, \
             tc.tile_pool(name="hpool", bufs=2) as hpool, \
             tc.tile_pool(name="fpool", bufs=2) as fpool, \
             tc.tile_pool(name="hps", bufs=2, space="PSUM") as hps, \
             tc.tile_pool(name="rps", bufs=1, space="PSUM") as rps, \
             tc.tile_pool(name="fps", bufs=1, space="PSUM") as fps, \
             tc.tile_pool(name="aps", bufs=1, space="PSUM") as aps:

            # ---- constants: two packed DMAs ----
            wpk_sb = const.tile([P, NW * H], bf16)
            nc.sync.dma_start(out=wpk_sb[:], in_=wpk[:])
            bpk_sb = const.tile([P, NW], f32)
            nc.sync.dma_start(out=bpk_sb[:], in_=bpk[:])
            wres_sb = wpk_sb[:, 0:H]
            wfc_sb = wpk_sb[:, (NW - 1) * H:NW * H]
            bres_sb = bpk_sb[:, 0:1]
            bfc_sb = bpk_sb[:, NW - 1:NW]
            macc = const.tile([P, GPC * 4], f32)

            def dma_inputs(g):
                """Both per-graph DMAs on the SP queue, features first, so
                the serial DMA-engine pool serves them in need order.
                Graph 0's A^T is quartered so its first DoubleRow pairs can
                start before the full 4MB lands."""
                xT = inpool.tile([P, N], bf16, tag="xin", name=f"x0_{g}")
                nc.sync.dma_start(out=xT[:], in_=x0[g * P:(g + 1) * P, :])
                if g == 0:
                    ats = []
                    for qq in range(4):
                        t = apool.tile([P, 4, N], f8, tag=f"atq{qq}",
                                       name=f"at0_{qq}")
                        nc.sync.dma_start(
                            out=t[:],
                            in_=at[0:P, qq * 4 * N:(qq + 1) * 4 * N].rearrange(
                                "p (s n) -> p s n", s=4))
                        ats.append(t)
                else:
                    t = apool.tile([P, NCH, N], f8, tag="at", name=f"at{g}")
                    nc.sync.dma_start(
                        out=t[:],
                        in_=at[g * P:(g + 1) * P, :].rearrange(
                            "p (s n) -> p s n", s=NCH))
                    ats = [t]
                return xT, ats

            def at_pair(ats, j, q):
                if len(ats) == 4:
                    t, jj = ats[j // 2], (j % 2) * 2
                else:
                    t, jj = ats[0], 2 * j
                return t[:, jj:jj + 2, q * 512:(q + 1) * 512]

            def emit_res_q(g, xT, x1T, q):
                """Residual strip q for graph g (PE filler work)."""
                rp = rps.tile([P, 512], f32, tag="rps", name=f"rp{g}_{q}")
                nc.tensor.matmul(out=rp[:], lhsT=wres_sb,
                                 rhs=xT[:, q * 512:(q + 1) * 512],
                                 start=True, stop=True)
                nc.scalar.activation(
                    out=x1T[:, q * 512:(q + 1) * 512], in_=rp[:],
                    func=RELU, bias=bres_sb)

            def emit_hgrp_q(g, l, x_src, h8t, q):
                """h-group q of layer l (chunks 4q..4q+3) + fp8 bulk cast."""
                hp = hps.tile([P, 512], f32, tag="hps", name=f"hp{g}_{l}_{q}")
                for c in range(4):
                    j = q * 4 + c
                    nc.tensor.matmul(
                        out=hp[:, c * H:(c + 1) * H],
                        lhsT=x_src[:, j * P:(j + 1) * P],
                        rhs=wpk_sb[:, (1 + l) * H:(2 + l) * H],
                        start=(c == 0), stop=(c == 3))
                nc.vector.tensor_copy(
                    out=h8t[:, q * 4:(q + 1) * 4, :].rearrange(
                        "p s f -> p (s f)"),
                    in_=hp[:])

            # ---- graph 0 prologue: inputs + residual + layer-0 h ----
            xT, ats = dma_inputs(0)
            x1T = x1pool.tile([P, N], bf16, tag="x1", name="x1_0")
            h8 = hpool.tile([P, NCH, H], f8, tag="h", name="h0_0")
            for q in range(4):
                emit_res_q(0, xT, x1T, q)
                emit_hgrp_q(0, 0, xT, h8, q)

            for g in range(GPC):
                if g + 1 < GPC:
                    xT_n, ats_n = dma_inputs(g + 1)
                for l in range(n_layers):
                    last = (l == n_layers - 1)
                    ps_l = [aps.tile([P, 512], f32, tag=f"aps{q}",
                                     name=f"as{g}_{l}_{q}") for q in range(4)]
                    # pairs 0-5 interleaved across strips: tolerates the
                    # trailing h-casts of this layer still landing
                    for j in range(6):
                        hj = h8[:, 2 * j:2 * j + 2, :]
                        for q in range(4):
                            nc.tensor.matmul(
                                out=ps_l[q][:], lhsT=hj,
                                rhs=at_pair(ats, j, q),
                                start=(j == 0), stop=False, perf_mode=DR)
                    # finish strips one at a time; each strip's relu output
                    # immediately feeds next-layer h (or fc + next-graph
                    # prologue filler on the last layer)
                    xn = xpool.tile([P, N], bf16, tag="x", name=f"x{g}_{l}")
                    if not last:
                        h8n = hpool.tile([P, NCH, H], f8, tag="h",
                                         name=f"h{g}_{l + 1}")
                    elif g + 1 < GPC:
                        x1T_n = x1pool.tile([P, N], bf16, tag="x1",
                                            name=f"x1_{g + 1}")
                        h8n = hpool.tile([P, NCH, H], f8, tag="h",
                                         name=f"h{g + 1}_0")
                    for q in range(4):
                        for j in (6, 7):
                            nc.tensor.matmul(
                                out=ps_l[q][:],
                                lhsT=h8[:, 2 * j:2 * j + 2, :],
                                rhs=at_pair(ats, j, q),
                                start=False, stop=(j == 7), perf_mode=DR)
                        nc.scalar.activation(
                            out=xn[:, q * 512:(q + 1) * 512], in_=ps_l[q][:],
                            func=RELU, bias=bpk_sb[:, 1 + l:2 + l])
                        if not last:
                            emit_hgrp_q(g, l + 1, xn, h8n, q)
                        else:
                            if g + 1 < GPC:
                                emit_res_q(g + 1, xT_n, x1T_n, q)
                                emit_hgrp_q(g + 1, 0, xT_n, h8n, q)
                            fp = fps.tile([P, 512], f32, tag="fps",
                                          name=f"fp{g}_{q}")
                            nc.tensor.matmul(
                                out=fp[:], lhsT=wfc_sb,
                                rhs=xn[:, q * 512:(q + 1) * 512],
                                start=True, stop=False)
                            nc.tensor.matmul(
                                out=fp[:], lhsT=wfc_sb,
                                rhs=x1T[:, q * 512:(q + 1) * 512],
                                start=False, stop=True)
                            fcq = fpool.tile([P, 512], f32, tag="fcq",
                                             name=f"fc{g}_{q}")
                            nc.scalar.activation(
                                out=fcq[:], in_=fp[:],
                                func=RELU, bias=bfc_sb,
                                accum_out=macc[:, g * 4 + q:g * 4 + q + 1])
                    h8 = h8n
                if g + 1 < GPC:
                    xT, ats, x1T = xT_n, ats_n, x1T_n

            nc.sync.dma_start(out=macc_out[:], in_=macc[:])

    nc.compile()
    return nc


class _Runner:
    """Compile once, keep the jitted sharded executable for repeat calls."""

    def __init__(self, n_layers: int):
        import jax
        from jax.sharding import Mesh, PartitionSpec
        from jax.experimental.shard_map import shard_map

        self.jax = jax
        nc = _build_program(n_layers)
        self.nc = nc
        bass2jax.install_neuronx_cc_hook()

        in_names, out_names, out_avals, zero_outs = [], [], [], []
        pid_name = nc.partition_id_tensor.name if nc.partition_id_tensor else None
        for alloc in nc.m.functions[0].allocations:
            if not isinstance(alloc, mybir.MemoryLocationSet):
                continue
            name = alloc.memorylocations[0].name
            if alloc.kind == "ExternalInput":
                if name != pid_name:
                    in_names.append(name)
            elif alloc.kind == "ExternalOutput":
                out_names.append(name)
                shape = tuple(alloc.tensor_shape)
                dtype = mybir.dt.np(alloc.dtype)
                out_avals.append(jax.core.ShapedArray(shape, dtype))
                zero_outs.append(np.zeros(shape, dtype))
        self.in_names = list(in_names)
        self.out_names = out_names
        self.zero_outs = zero_outs
        n_params = len(in_names)
        all_names = in_names + out_names + ([pid_name] if pid_name else [])

        def _body(*args):
            operands = list(args)
            if pid_name is not None:
                operands.append(bass2jax.partition_id_tensor())
            return tuple(bass2jax._bass_exec_p.bind(
                *operands,
                out_avals=tuple(out_avals),
                in_names=tuple(all_names),
                out_names=tuple(out_names),
                lowering_input_output_aliases=(),
                sim_require_finite=True,
                sim_require_nnan=True,
                nc=nc,
            ))

        devices = jax.devices()[:N_CORES]
        mesh = Mesh(np.asarray(devices), ("core",))
        self.fn = jax.jit(
            shard_map(_body, mesh=mesh,
                      in_specs=(PartitionSpec("core"),) * (n_params + len(out_names)),
                      out_specs=(PartitionSpec("core"),) * len(out_names),
                      check_rep=False),
            keep_unused=True)

    def run(self, concat_inputs: list[np.ndarray]):
        jax = self.jax
        concat_zeros = [np.zeros((N_CORES * z.shape[0], *z.shape[1:]), z.dtype)
                        for z in self.zero_outs]
        outs = self.fn(*concat_inputs, *concat_zeros)
        jax.block_until_ready(outs)
        return {name: np.asarray(outs[i]) for i, name in enumerate(self.out_names)}


_RUNNERS: dict[int, _Runner] = {}


def _prepare_inputs(all_features, feature_index, edge_index,
                    lin_res_w, lin_res_b, gcn_w, gcn_b,
                    fc1_w, fc1_b, lin_w, lin_b, n_layers):
    """Build the concatenated (over cores, axis 0) device input list."""
    feats = np.asarray(all_features, np.float32)
    fi = np.asarray(feature_index).astype(np.int64)
    ei = np.asarray(edge_index).astype(np.int32)

    # host gather + transpose to feature-major bf16 [G, 128, 2048]
    x0_all = np.ascontiguousarray(
        feats[fi].transpose(0, 2, 1)).astype(BFNP)          # [G, D, N]

    # A^T per graph: accumulate duplicate (src,dst) cells, quantize fp8,
    # swizzle to [128 part, 16 chunk, 2048 dst].
    at_all = np.zeros((G, N * N), np.float32)
    diag_keys = (np.arange(N, dtype=np.int64) * (N + 1)).astype(np.int32)
    for g in range(G):
        src = ei[g, 0]
        dst = ei[g, 1]
        deg = np.bincount(dst, minlength=N).astype(np.float32) + 1.0
        dinv = 1.0 / np.sqrt(deg)
        coef = dinv[src] * dinv[dst]
        keys = np.concatenate([src.astype(np.int32) * N + dst, diag_keys])
        vals = np.concatenate([coef, dinv * dinv]).astype(np.float64)
        order = np.argsort(keys, kind="stable")
        ks, vs = keys[order], vals[order]
        first = np.empty(len(ks), bool)
        first[0] = True
        first[1:] = ks[1:] != ks[:-1]
        starts = np.nonzero(first)[0]
        sums = np.add.reduceat(vs, starts).astype(np.float32)
        np.put(at_all[g], ks[starts], sums)
    at8 = at_all.reshape(G, NCH, P, N).transpose(0, 2, 1, 3)  # [G,128,16,2048]
    at8 = np.ascontiguousarray(at8).astype(E4NP).reshape(G, P, NCH * N)

    # packed weights [128, (2+L)*128] bf16: wres | gw[0..L) | wfc
    NW = 2 + n_layers
    wpk = np.empty((P, NW * H), BFNP)
    wpk[:, 0:H] = np.asarray(lin_res_w, np.float32).astype(BFNP)
    for l in range(n_layers):
        wpk[:, (1 + l) * H:(2 + l) * H] = (
            np.asarray(gcn_w[l], np.float32).astype(BFNP))
    wpk[:, (NW - 1) * H:] = np.asarray(fc1_w, np.float32).astype(BFNP)
    # packed biases [128, 2+L] f32: bres | gb[0..L) | bfc
    bpk = np.empty((P, NW), np.float32)
    bpk[:, 0] = np.asarray(lin_res_b, np.float32)
    for l in range(n_layers):
        bpk[:, 1 + l] = np.asarray(gcn_b[l], np.float32)
    bpk[:, NW - 1] = np.asarray(fc1_b, np.float32)

    per_core = {}
    per_core["x0"] = [x0_all[c * GPC:(c + 1) * GPC].reshape(GPC * P, N)
                      for c in range(N_CORES)]
    per_core["at"] = [at8[c * GPC:(c + 1) * GPC].reshape(GPC * P, NCH * N)
                      for c in range(N_CORES)]
    per_core["wpk"] = [wpk] * N_CORES
    per_core["bpk"] = [bpk] * N_CORES
    return per_core


def kernel(all_features, feature_index, edge_index, action,
           lin_res_w, lin_res_b, gcn_w, gcn_b,
           fc1_w, fc1_b, lin_w, lin_b):
    n_layers = int(action) + 1
    assert 1 <= n_layers <= 3

    if n_layers not in _RUNNERS:
        _RUNNERS[n_layers] = _Runner(n_layers)
    runner = _RUNNERS[n_layers]

    per_core = _prepare_inputs(
        all_features, feature_index, edge_index,
        lin_res_w, lin_res_b, gcn_w, gcn_b, fc1_w, fc1_b, lin_w, lin_b,
        n_layers)

    concat = [np.concatenate(per_core[name], axis=0)
              for name in runner.in_names]
    outs = runner.run(concat)

    # host head: node-sums -> means -> logits -> log_softmax
    macc = outs["macc_out"].reshape(N_CORES, P, GPC, 4)
    means = macc.sum(axis=3).transpose(0, 2, 1).reshape(G, H) / N   # [G, H]
    lg = means @ np.asarray(lin_w, np.float32) + np.asarray(lin_b, np.float32)
    mx = lg.max(axis=1, keepdims=True)
    ls = lg - mx - np.log(np.exp(lg - mx).sum(axis=1, keepdims=True))
    return np.asarray(ls, np.float32), np.asarray(lg, np.float32)
